# revision 1
# baseline (speedup 1.0000x reference)
"""GPT-NeoX attention layer as a Bass/Tile kernel for 8 Trainium2 NeuronCores.

Problem: hidden[2048,1,4096] -> QKV proj (W[4096,12288]) -> 32-head attention
(head_dim 128, rotary on first 32 dims, causal) -> dense proj (W[4096,4096]).

Sharding: tensor-parallel over heads (4 heads/core). Each core:
  P0: PE-transposes its 512-column shard of hidden; AllGather -> full hidden^T.
  P1: QKV projection. q/k produced TRANSPOSED ([head_dim, seq], via a host-side
      column permutation of W_qkv so the rotary dims of the 4 heads stack into
      full 128-partition tiles); v produced in [seq, head_dim] layout. Rotary
      applied on-chip with host cos/sin tables (rotate_half via a
      partition-permuting SBUF->SBUF DMA, sign baked into the sin table).
  P2: attention per head: scores^T tiles [kv 128 x q 512] on PE (contraction =
      head_dim, one matmul per tile), additive causal mask on the 4 diagonal
      tiles, exp on ScalarE (no max-subtraction needed: scores are O(10)),
      denominator via ones-matmul partition reduction, PV matmul accumulates
      ctx^T; normalization by 1/denom broadcast through a rank-1 matmul.
  P3: AllGather ctx^T -> full [4096, 2048].
  P4: dense projection, column-sharded: out[:, c*512:(c+1)*512].
Host gathers by concatenating the 8 column slices.

dtypes: large streamed tensors (weights, hidden^T, ctx) are fp16 to halve DMA
traffic (PE rate is identical: 1 cycle/row for fp16 and f32r); all attention
math (q/k/v tiles, exp, denominators) is f32r (fp32 with 11-bit mantissa,
TF32-like, full PE rate at free-dim >= 256); softmax/psum accumulation fp32.
"""
import sys
import os

sys.path.insert(0, "/opt/trn_rl_repo")

import numpy as np

import concourse.bacc as bacc
import concourse.mybir as mybir
import concourse.tile as tile

SEQ = 2048
HIDDEN = 4096
HEADS = 32
HD = 128
ROT = 32
HALF = ROT // 2  # 16
N_CORES = 8
HPC = HEADS // N_CORES       # 4 heads per core
CW = HPC * HD                # 512 columns of work per core (v / ctx / dense out)
KT = HIDDEN // 128           # 32 k-tiles over the hidden dim
SB = 512                     # sequence block for QKV + attention i-blocks
NSB = SEQ // SB              # 4
NST = SEQ // 128             # 16 sequence tiles
NEG = -1.0e9                 # additive mask value (pre-scale)
SCALE = float(1.0 / np.sqrt(HD))

F32 = mybir.dt.float32
F32R = mybir.dt.float32r
F16 = mybir.dt.float16
AF = mybir.ActivationFunctionType

_CACHE = {}


def _f32(ap):
    return ap.bitcast(F32)


def _build_program(rep=1, trace_sim=False, skip_cc=False, phases="all"):
    nc = bacc.Bacc("TRN2", target_bir_lowering=False, debug=False,
                   num_devices=N_CORES)

    # ---- I/O ---------------------------------------------------------------
    hid_shard = nc.dram_tensor("hid_shard", [SEQ, CW], F32, kind="ExternalInput")
    # w_qk: [m_tile, k_tile, 128, 128] fp16, column-permuted (see _host_prep)
    w_qk = nc.dram_tensor("w_qk", [8, KT, 128, 128], F16, kind="ExternalInput")
    w_v = nc.dram_tensor("w_v", [KT, 128, CW], F16, kind="ExternalInput")
    w_d = nc.dram_tensor("w_d", [KT, 128, CW], F16, kind="ExternalInput")
    b_qk = nc.dram_tensor("b_qk", [128, 8], F32, kind="ExternalInput")
    b_v = nc.dram_tensor("b_v", [1, CW], F16, kind="ExternalInput")
    b_d = nc.dram_tensor("b_d", [1, CW], F16, kind="ExternalInput")
    cos_in = nc.dram_tensor("cos_in", [128, SEQ], F32, kind="ExternalInput")
    sin_in = nc.dram_tensor("sin_in", [128, SEQ], F32, kind="ExternalInput")
    mask_in = nc.dram_tensor("mask_in", [128, 4 * SB], F32, kind="ExternalInput")
    ident_in = nc.dram_tensor("ident_in", [128, 128], F32, kind="ExternalInput")
    ones_col_in = nc.dram_tensor("ones_col_in", [128, 1], F32R,
                                 kind="ExternalInput")
    ones_row_in = nc.dram_tensor("ones_row_in", [1, 128], F32R,
                                 kind="ExternalInput")
    ones_row16_in = nc.dram_tensor("ones_row16_in", [1, 128], F16,
                                   kind="ExternalInput")
    out = nc.dram_tensor("out", [SEQ, CW], F32, kind="ExternalOutput")

    rg = [list(range(N_CORES))]

    with tile.TileContext(nc, trace_sim=trace_sim) as tc:
        with (
            tc.tile_pool(name="const", bufs=1) as constp,
            tc.tile_pool(name="dram", bufs=1, space="DRAM") as dramp,
        ):
            # constants
            ident = constp.tile([128, 128], F32)
            ones_col = constp.tile([128, 1], F32R)
            ones_row = constp.tile([1, 128], F32R)
            ones_row16 = constp.tile([1, 128], F16)
            bqk_sb = constp.tile([128, 8], F32)
            bv_sb = constp.tile([1, CW], F16)
            bd_sb = constp.tile([1, CW], F16)
            cos_sb = constp.tile([128, SEQ], F32)
            sin_sb = constp.tile([128, SEQ], F32)
            nc.sync.dma_start(ident[:], ident_in[:])
            nc.sync.dma_start(ones_col[:], ones_col_in[:])
            nc.sync.dma_start(ones_row[:], ones_row_in[:])
            nc.sync.dma_start(ones_row16[:], ones_row16_in[:])
            nc.sync.dma_start(bqk_sb[:], b_qk[:])
            nc.sync.dma_start(bv_sb[:], b_v[:])
            nc.sync.dma_start(bd_sb[:], b_d[:])
            nc.sync.dma_start(cos_sb[:], cos_in[:])
            nc.sync.dma_start(sin_sb[:], sin_in[:])

            for _rep in range(rep):
              # collective bounce buffers, one per sequence block so each
              # AllGather chunk can overlap compute (fresh per rep)
              ccin_h = [dramp.tile([CW, SB], F16, name=f"ccin_h{_rep}_{i}")
                        for i in range(NSB)]
              ccout_h = [dramp.tile([HIDDEN, SB], F16, addr_space="Shared",
                                    name=f"ccout_h{_rep}_{i}")
                         for i in range(NSB)]
              ccin_ctx = [dramp.tile([CW, SB], F16, name=f"ccin_ctx{_rep}_{i}")
                          for i in range(NSB)]
              ccout_ctx = [dramp.tile([HIDDEN, SB], F16, addr_space="Shared",
                                      name=f"ccout_ctx{_rep}_{i}")
                           for i in range(NSB)]

              # ---- P0: transpose own shard of hidden, AllGather -------------
              with (
                  tc.tile_pool(name="p0sb", bufs=6) as p0sb,
                  tc.tile_pool(name="p0ps", bufs=2, space="PSUM") as p0ps,
              ):
                  for sb in range(NSB):
                      for st4 in range(4):
                          st = sb * 4 + st4
                          hs_t = p0sb.tile([128, CW], F32, name="hs_t")
                          nc.sync.dma_start(
                              hs_t[:], hid_shard[st * 128:(st + 1) * 128, :])
                          ht_t = p0sb.tile([128, CW], F16, name="ht_t")
                          for kb in range(CW // 128):
                              tp = p0ps.tile([128, 128], F32, name="tp")
                              nc.tensor.transpose(
                                  tp[:], hs_t[:, kb * 128:(kb + 1) * 128],
                                  ident[:])
                              nc.scalar.activation(
                                  ht_t[:, kb * 128:(kb + 1) * 128], tp[:],
                                  AF.Copy)
                          # batched write: [128, (kb c)] -> rows kb*128+c
                          nc.gpsimd.dma_start(
                              ccin_h[sb][:, st4 * 128:(st4 + 1) * 128].rearrange(
                                  "(kb c) s -> c kb s", kb=4),
                              ht_t[:].rearrange("c (kb s) -> c kb s", kb=4))
                      if not skip_cc:
                          nc.gpsimd.collective_compute(
                              "AllGather", mybir.AluOpType.bypass,
                              replica_groups=rg,
                              ins=[ccin_h[sb][:].opt()],
                              outs=[ccout_h[sb][:].opt()])

              # persistent QKV outputs (live through P1+P2)
              with tc.tile_pool(name="qkvout", bufs=1) as qkvp:
                  qh = [qkvp.tile([128, SEQ], F32R, name=f"qh{h}")
                        for h in range(HPC)]
                  kh = [qkvp.tile([128, SEQ], F32R, name=f"kh{h}")
                        for h in range(HPC)]
                  vsb = [qkvp.tile([128, CW], F32R, name=f"v{s}")
                         for s in range(NST)]

                  # ---- P1: QKV projection ----------------------------------
                  with (
                      tc.tile_pool(name="htp", bufs=10) as htp,
                      tc.tile_pool(name="wqp", bufs=4) as wqp,
                      tc.tile_pool(name="wvp", bufs=2) as wvp,
                      tc.tile_pool(name="rotp", bufs=2) as rotp,
                      tc.tile_pool(name="rscp", bufs=6) as rscp,
                      tc.tile_pool(name="qkps", bufs=2, space="PSUM") as qkps,
                      tc.tile_pool(name="vps", bufs=4, space="PSUM") as vps,
                  ):
                      def rope(rot_t, dst, sb):
                          """rot_t: [128, SB], rows hl*32+d = rotary dim d of
                          head hl. rotate_half is materialized by a
                          partition-permuting SBUF->SBUF DMA; the sign lives in
                          the sin table."""
                          cs = cos_sb[:, sb * SB:(sb + 1) * SB]
                          sn = sin_sb[:, sb * SB:(sb + 1) * SB]
                          shf = rscp.tile([128, SB], F32R, name="rsc")
                          for hl in range(HPC):
                              r = hl * ROT
                              nc.gpsimd.dma_start(shf[r:r + HALF, :],
                                                  rot_t[r + HALF:r + ROT, :])
                              nc.gpsimd.dma_start(shf[r + HALF:r + ROT, :],
                                                  rot_t[r:r + HALF, :])
                          t1 = rscp.tile([128, SB], F32R, name="rsc")
                          t2 = rscp.tile([128, SB], F32R, name="rsc")
                          rp = rscp.tile([128, SB], F32R, name="rsc")
                          nc.vector.tensor_mul(t1[:], _f32(rot_t[:]), cs)
                          nc.vector.tensor_mul(t2[:], _f32(shf[:]), sn)
                          nc.vector.tensor_add(rp[:], _f32(t1[:]), _f32(t2[:]))
                          for hl in range(HPC):
                              nc.scalar.activation(
                                  dst[hl][0:ROT, sb * SB:(sb + 1) * SB],
                                  rp[hl * ROT:(hl + 1) * ROT, :], AF.Copy)

                      def evac_qk(m, pq, sb):
                          scols = slice(sb * SB, (sb + 1) * SB)
                          if m == 0 or m == 1:
                              rot_t = rotp.tile([128, SB], F32R, name="rot_t")
                              nc.scalar.activation(rot_t[:], pq[:], AF.Identity,
                                                   bias=bqk_sb[:, m:m + 1])
                              rope(rot_t, qh if m == 0 else kh, sb)
                          else:
                              # 32-row chunks: compute-engine partition accesses
                              # >32 rows must start at partition 0; head spans
                              # (96 rows) are exactly 3 chunks.
                              t = (m - 2) % 3
                              dst = qh if m <= 4 else kh
                              for ch in range(4):
                                  g = t * 128 + ch * 32
                                  hl = g // 96
                                  dlo = 32 + g - hl * 96
                                  nc.scalar.activation(
                                      dst[hl][dlo:dlo + 32, scols],
                                      pq[ch * 32:(ch + 1) * 32, :], AF.Identity,
                                      bias=bqk_sb[ch * 32:(ch + 1) * 32,
                                                  m:m + 1])

                      for sb in range(NSB):
                          scols = slice(sb * SB, (sb + 1) * SB)
                          # hidden^T k-tiles for this s-block, resident: 8 tiles
                          # of [128, 4*SB] fp16, each loaded with one DMA
                          ht4 = []
                          for kg in range(8):
                              h4 = htp.tile([128, 4 * SB], F16, name="ht4")
                              nc.sync.dma_start(
                                  h4[:].rearrange("p (k s) -> p k s", k=4),
                                  ccout_h[sb][kg * 512:(kg + 1) * 512,
                                              :].rearrange(
                                      "(k p) s -> p k s", k=4))
                              ht4.append(h4)

                          def htk(k):
                              return ht4[k // 4][:, (k % 4) * SB:
                                                 (k % 4 + 1) * SB]

                          def v_part():
                              # k-outer, 4 psum banks held over the k sweep
                              pv = [vps.tile([128, CW], F32, name="pv")
                                    for _ in range(4)]
                              for k in range(KT):
                                  if k % 4 == 0:
                                      wvb = wvp.tile([128, 4 * CW], F16,
                                                     name="wvb")
                                      nc.sync.dma_start(
                                          wvb[:].rearrange(
                                              "p (k c) -> p k c", k=4),
                                          w_v[k:k + 4].rearrange(
                                              "k p c -> p k c"))
                                  hk = htk(k)
                                  for q4 in range(4):
                                      nc.tensor.matmul(
                                          pv[q4][:],
                                          hk[:, q4 * 128:(q4 + 1) * 128],
                                          wvb[:, (k % 4) * CW:(k % 4 + 1) * CW],
                                          start=(k == 0), stop=False)
                              for q4 in range(4):
                                  nc.tensor.matmul(pv[q4][:], ones_row16[:],
                                                   bv_sb[:], start=False,
                                                   stop=True)
                                  nc.scalar.activation(vsb[sb * 4 + q4][:],
                                                       pv[q4][:], AF.Copy)

                          def qk_part():
                              # m-outer, k-inner; W strips batched (16 k/DMA)
                              for m in range(8):
                                  pq = qkps.tile([128, SB], F32, name="pq")
                                  for k in range(KT):
                                      if k % 16 == 0:
                                          wqb = wqp.tile([128, 16 * 128], F16,
                                                         name="wqb")
                                          nc.sync.dma_start(
                                              wqb[:].rearrange(
                                                  "p (k c) -> p k c", k=16),
                                              w_qk[m, k:k + 16].rearrange(
                                                  "k p c -> p k c"))
                                      nc.tensor.matmul(
                                          pq[:],
                                          wqb[:, (k % 16) * 128:
                                              (k % 16 + 1) * 128],
                                          htk(k), start=(k == 0),
                                          stop=(k == KT - 1))
                                  evac_qk(m, pq, sb)

                          v_part()
                          qk_part()

                  # ---- W_dense prefetch + P2 + P4 (wdp pool spans both so
                  # the dense weights stream in during attention) -----------
                  wdp_ctx = tc.tile_pool(name="wdp", bufs=1)
                  wdp = wdp_ctx.__enter__()
                  wd_sb = []
                  if phases == "all":
                      for k in range(KT):
                          w_t = wdp.tile([128, CW], F16, name=f"wd{k}")
                          nc.sync.dma_start(w_t[:], w_d[k].opt())
                          wd_sb.append(w_t)

                  # ---- P2: attention ---------------------------------------
                  if phases == "p01":
                      for h in range(HPC):
                          nc.sync.dma_start(
                              ccin_ctx[0][h * 128:(h + 1) * 128, :],
                              qh[h][:, 0:SB // 2].bitcast(F16))
                      nc.sync.dma_start(out[0:128, :], _f32(vsb[0][:]))
                  if phases != "p01":
                   with (
                       tc.tile_pool(name="maskp", bufs=1) as maskp,
                       tc.tile_pool(name="exp", bufs=6) as exp_p,
                       tc.tile_pool(name="accp", bufs=3) as accp,
                       tc.tile_pool(name="rcp", bufs=3) as rcp,
                       tc.tile_pool(name="rbp", bufs=3) as rbp,
                       tc.tile_pool(name="ctxp", bufs=3) as ctxp,
                       tc.tile_pool(name="sps", bufs=2, space="PSUM") as sps,
                       tc.tile_pool(name="cps", bufs=2, space="PSUM") as cps,
                       tc.tile_pool(name="dps", bufs=1, space="PSUM") as dps,
                       tc.tile_pool(name="rbps", bufs=1, space="PSUM") as rbps,
                   ):
                       mask_sb = maskp.tile([128, 4 * SB], F32)
                       nc.sync.dma_start(mask_sb[:], mask_in[:])

                       for ib in range(NSB):
                           for h in range(HPC):
                               icols = slice(ib * SB, (ib + 1) * SB)
                               njt = 4 * (ib + 1)
                               cp = cps.tile([128, SB], F32, name="cp")
                               acc = accp.tile([128, SB], F32R, name="acc")
                               for jp in range(njt // 2):
                                   # two j-tiles share one [128, 2*SB] psum so
                                   # exp and the denominator add run once per
                                   # pair (ACT is the P2 bottleneck)
                                   sp = sps.tile([128, 2 * SB], F32, name="sp")
                                   for u in range(2):
                                       jt = 2 * jp + u
                                       nc.tensor.matmul(
                                           sp[:, u * SB:(u + 1) * SB],
                                           kh[h][:, jt * 128:(jt + 1) * 128],
                                           qh[h][:, icols], start=True,
                                           stop=True)
                                   if 2 * jp + 1 >= 4 * ib:
                                       t = 2 * jp - 4 * ib
                                       nc.vector.tensor_add(
                                           sp[:], sp[:],
                                           mask_sb[:, t * SB:(t + 2) * SB])
                                   ex = exp_p.tile([128, 2 * SB], F32R,
                                                   name="ex")
                                   nc.scalar.activation(ex[:], sp[:], AF.Exp,
                                                        scale=SCALE)
                                   if jp == 0:
                                       nc.vector.tensor_add(
                                           acc[:], _f32(ex[:, 0:SB]),
                                           _f32(ex[:, SB:2 * SB]))
                                   else:
                                       nc.vector.tensor_add(
                                           acc[:], _f32(acc[:]),
                                           _f32(ex[:, 0:SB]))
                                       nc.vector.tensor_add(
                                           acc[:], _f32(acc[:]),
                                           _f32(ex[:, SB:2 * SB]))
                                   for u in range(2):
                                       jt = 2 * jp + u
                                       nc.tensor.matmul(
                                           cp[:],
                                           vsb[jt][:, h * 128:(h + 1) * 128],
                                           ex[:, u * SB:(u + 1) * SB],
                                           start=(jt == 0),
                                           stop=(jt == njt - 1))
                               dn = dps.tile([1, SB], F32, name="dn")
                               nc.tensor.matmul(dn[:], ones_col[:], acc[:],
                                                start=True, stop=True)
                               rc = rcp.tile([1, SB], F32R, name="rc")
                               with nc.allow_low_precision(
                                       reason="f32r: 11-bit mantissa is plenty "
                                              "for the softmax denominator"):
                                   nc.vector.reciprocal(rc[:], dn[:])
                               rb = rbps.tile([128, SB], F32, name="rb")
                               nc.tensor.matmul(rb[:], ones_row[:], rc[:],
                                                start=True, stop=True)
                               rbs = rbp.tile([128, SB], F32R, name="rbs")
                               nc.scalar.activation(rbs[:], rb[:], AF.Copy)
                               ctxn = ctxp.tile([128, SB], F16, name="ctxn")
                               nc.vector.tensor_mul(ctxn[:], cp[:], _f32(rbs[:]))
                               nc.gpsimd.dma_start(
                                   ccin_ctx[ib][h * 128:(h + 1) * 128, :],
                                   ctxn[:])
                           if not skip_cc and phases == "all":
                               nc.gpsimd.collective_compute(
                                   "AllGather", mybir.AluOpType.bypass,
                                   replica_groups=rg,
                                   ins=[ccin_ctx[ib][:].opt()],
                                   outs=[ccout_ctx[ib][:].opt()])

              # ---- P3: ctx AllGathers fired per i-block inside P2 ----------

                  # ---- P4: dense projection (column shard) ---------------------
                  if phases == "p012":
                      nc.sync.dma_start(out[0:CW, 0:SB // 2],
                                        ccin_ctx[0][0:CW, :].bitcast(F32))
                  if phases == "all":
                   with (
                       tc.tile_pool(name="ctp", bufs=5) as ctp,
                       tc.tile_pool(name="outp", bufs=3) as outp,
                       tc.tile_pool(name="pdps", bufs=5, space="PSUM") as pdps,
                   ):
                       for mq in range(4):
                           pd = [pdps.tile([128, CW], F32, name="pd")
                                 for _ in range(4)]
                           for k in range(KT):
                               k4 = k % 4
                               if k4 == 0:
                                   ct4 = ctp.tile([128, 4 * SB], F16, name="ct4")
                                   nc.sync.dma_start(
                                       ct4[:].rearrange("p (k s) -> p k s", k=4),
                                       ccout_ctx[mq][k * 128:(k + 4) * 128,
                                                     :].rearrange(
                                           "(k p) s -> p k s", k=4))
                               ct = ct4[:, k4 * SB:(k4 + 1) * SB]
                               for m4 in range(4):
                                   nc.tensor.matmul(
                                       pd[m4][:], ct[:, m4 * 128:(m4 + 1) * 128],
                                       wd_sb[k][:], start=(k == 0), stop=False)
                           for m4 in range(4):
                               nc.tensor.matmul(pd[m4][:], ones_row16[:], bd_sb[:],
                                                start=False, stop=True)
                               ot = outp.tile([128, CW], F32, name="ot")
                               nc.scalar.activation(ot[:], pd[m4][:], AF.Copy)
                               st = mq * 4 + m4
                               nc.sync.dma_start(out[st * 128:(st + 1) * 128, :],
                                                 ot[:])


                  wdp_ctx.__exit__(None, None, None)

    nc.compile()
    return nc


def _get_exec(rep=1):
    if ("exec", rep) in _CACHE:
        return _CACHE[("exec", rep)]
    import jax
    from jax.sharding import Mesh, PartitionSpec
    from jax.experimental.shard_map import shard_map
    from concourse import bass2jax

    nc = _build_program(rep=rep)
    bass2jax.install_neuronx_cc_hook()

    partition_name = (nc.partition_id_tensor.name
                      if nc.partition_id_tensor else None)
    in_names = []
    out_names = []
    out_avals = []
    zero_shapes = []
    for alloc in nc.m.functions[0].allocations:
        if not isinstance(alloc, mybir.MemoryLocationSet):
            continue
        name = alloc.memorylocations[0].name
        if alloc.kind == "ExternalInput":
            if name != partition_name:
                in_names.append(name)
        elif alloc.kind == "ExternalOutput":
            np_dt = mybir.dt.np(alloc.dtype)
            out_names.append(name)
            out_avals.append(
                jax.core.ShapedArray(tuple(alloc.tensor_shape), np_dt))
            zero_shapes.append((tuple(alloc.tensor_shape), np_dt))

    n_params = len(in_names)
    n_outs = len(out_names)
    all_in_names = in_names + out_names
    if partition_name is not None:
        all_in_names = all_in_names + [partition_name]
    donate = tuple(range(n_params, n_params + n_outs))

    def _body(*args):
        operands = list(args)
        if partition_name is not None:
            operands.append(bass2jax.partition_id_tensor())
        outs = bass2jax._bass_exec_p.bind(
            *operands,
            out_avals=tuple(out_avals),
            in_names=tuple(all_in_names),
            out_names=tuple(out_names),
            lowering_input_output_aliases=(),
            sim_require_finite=True,
            sim_require_nnan=True,
            nc=nc,
        )
        return tuple(outs)

    devices = jax.devices()[:N_CORES]
    mesh = Mesh(np.asarray(devices), ("core",))
    in_specs = (PartitionSpec("core"),) * (n_params + n_outs)
    out_specs = (PartitionSpec("core"),) * n_outs
    sharded = jax.jit(
        shard_map(_body, mesh=mesh, in_specs=in_specs, out_specs=out_specs,
                  check_rep=False),
        donate_argnums=donate, keep_unused=True)

    _CACHE[("nc", rep)] = nc
    _CACHE[("exec", rep)] = (sharded, in_names, out_names, out_avals,
                             zero_shapes)
    return _CACHE[("exec", rep)]


def _run_cores(in_maps):
    """Run the SPMD program; in_maps is a list of 8 dicts name->np.ndarray."""
    sharded, in_names, out_names, out_avals, zero_shapes = _get_exec()
    concat_in = [
        np.concatenate([np.asarray(in_maps[c][n]) for c in range(N_CORES)],
                       axis=0)
        for n in in_names
    ]
    concat_zeros = [
        np.zeros((N_CORES * s[0], *s[1:]), dt) for (s, dt) in zero_shapes
    ]
    out_arrs = sharded(*concat_in, *concat_zeros)
    return [
        {n: np.asarray(out_arrs[i]).reshape(N_CORES, *out_avals[i].shape)[c]
         for i, n in enumerate(out_names)}
        for c in range(N_CORES)
    ]


def benchmark(in_maps, iters=10, rep=1):
    """Time repeated executions with device-resident inputs. Returns list of
    per-call wall seconds (axon RPC overhead included)."""
    import time
    import jax
    import jax.numpy as jnp
    from jax.sharding import Mesh, PartitionSpec, NamedSharding

    sharded, in_names, out_names, out_avals, zero_shapes = _get_exec(rep)
    devices = jax.devices()[:N_CORES]
    mesh = Mesh(np.asarray(devices), ("core",))
    shard = NamedSharding(mesh, PartitionSpec("core"))
    dev_in = [
        jax.device_put(
            np.concatenate([np.asarray(in_maps[c][n]) for c in range(N_CORES)],
                           axis=0), shard)
        for n in in_names
    ]
    jax.block_until_ready(dev_in)

    def make_zeros():
        zs = [jnp.zeros((N_CORES * s[0], *s[1:]), dt, device=shard)
              for (s, dt) in zero_shapes]
        jax.block_until_ready(zs)
        return zs

    out = sharded(*dev_in, *make_zeros())
    jax.block_until_ready(out)
    times = []
    for _ in range(iters):
        zs = make_zeros()
        t0 = time.perf_counter()
        out = sharded(*dev_in, *zs)
        jax.block_until_ready(out)
        times.append(time.perf_counter() - t0)
    return times


def _host_prep(hidden_states, W_qkv, b_qkv, W_dense, b_dense):
    hid = np.ascontiguousarray(
        np.asarray(hidden_states, dtype=np.float32).reshape(SEQ, HIDDEN))
    W_qkv = np.asarray(W_qkv, dtype=np.float32)
    b_qkv = np.asarray(b_qkv, dtype=np.float32)
    W_dense = np.asarray(W_dense, dtype=np.float32)
    b_dense = np.asarray(b_dense, dtype=np.float32)

    # rotary tables, computed in float32 exactly as the reference does
    inv_freq = (1.0 / (np.float32(10000.0) **
                       (np.arange(0, ROT, 2, dtype=np.float32)
                        / np.float32(ROT))))
    t = np.arange(SEQ, dtype=np.float32)
    freqs = t[:, None] * inv_freq[None, :]          # [SEQ, 16]
    cosf = np.cos(freqs).T                          # [16, SEQ]
    sinf = np.sin(freqs).T
    # row hl*32 + d: cos(emb[d mod 16]); sin carries the rotate_half sign
    cos_blk = np.concatenate([cosf, cosf], axis=0)      # [32, SEQ]
    sin_blk = np.concatenate([-sinf, sinf], axis=0)
    cos_t = np.tile(cos_blk, (HPC, 1)).astype(np.float32)  # [128, SEQ]
    sin_t = np.tile(sin_blk, (HPC, 1)).astype(np.float32)

    # additive causal masks for the 4 diagonal j-tiles of each i-block
    pj = np.arange(128)[:, None]
    fi = np.arange(SB)[None, :]
    mask = np.concatenate(
        [np.where(128 * t_ + pj <= fi, 0.0, NEG) for t_ in range(4)],
        axis=1).astype(np.float32)                   # [128, 4*SB]

    ident = np.eye(128, dtype=np.float32)

    in_maps = []
    for c in range(N_CORES):
        heads = [HPC * c + i for i in range(HPC)]
        qcol = lambda h, d: h * 3 * HD + d
        kcol = lambda h, d: h * 3 * HD + HD + d
        vcol = lambda h, d: h * 3 * HD + 2 * HD + d
        perm = []
        perm += [qcol(h, d) for h in heads for d in range(ROT)]
        perm += [kcol(h, d) for h in heads for d in range(ROT)]
        perm += [qcol(h, d) for h in heads for d in range(ROT, HD)]
        perm += [kcol(h, d) for h in heads for d in range(ROT, HD)]
        perm = np.asarray(perm)
        vperm = np.asarray([vcol(h, d) for h in heads for d in range(HD)])

        w_qk = W_qkv[:, perm].astype(np.float16)     # [4096, 1024]
        w_qk = np.ascontiguousarray(
            w_qk.reshape(KT, 128, 8, 128).transpose(2, 0, 1, 3))
        w_v = np.ascontiguousarray(
            W_qkv[:, vperm].astype(np.float16).reshape(KT, 128, CW))
        w_d = np.ascontiguousarray(
            W_dense[:, c * CW:(c + 1) * CW].astype(np.float16).reshape(
                KT, 128, CW))
        in_maps.append({
            "hid_shard": np.ascontiguousarray(hid[:, c * CW:(c + 1) * CW]),
            "w_qk": w_qk,
            "w_v": w_v,
            "w_d": w_d,
            "b_qk": np.ascontiguousarray(b_qkv[perm].reshape(8, 128).T),
            "b_v": b_qkv[vperm].astype(np.float16).reshape(1, CW),
            "b_d": (b_dense[c * CW:(c + 1) * CW].astype(np.float16)
                    .reshape(1, CW)),
            "cos_in": cos_t,
            "sin_in": sin_t,
            "mask_in": mask,
            "ident_in": ident,
            "ones_col_in": np.ones((128, 1), np.float32),
            "ones_row_in": np.ones((1, 128), np.float32),
            "ones_row16_in": np.ones((1, 128), np.float16),
        })
    return in_maps


def kernel(hidden_states, attention_mask=None, W_qkv=None, b_qkv=None,
           W_dense=None, b_dense=None, **_unused):
    in_maps = _host_prep(hidden_states, W_qkv, b_qkv, W_dense, b_dense)
    results = _run_cores(in_maps)
    full = np.concatenate([results[c]["out"] for c in range(N_CORES)], axis=1)
    return full.reshape(SEQ, 1, HIDDEN).astype(np.float32)


if __name__ == "__main__":
    rng = np.random.default_rng(0)
    ins = {
        "hidden_states": rng.standard_normal((SEQ, 1, HIDDEN),
                                             dtype=np.float32),
        "attention_mask": np.triu(np.ones((SEQ, SEQ), dtype=bool),
                                  1)[None, None],
        "W_qkv": (rng.standard_normal((HIDDEN, 3 * HIDDEN), dtype=np.float32)
                  * 0.02),
        "b_qkv": np.zeros(3 * HIDDEN, np.float32),
        "W_dense": (rng.standard_normal((HIDDEN, HIDDEN), dtype=np.float32)
                    * 0.02),
        "b_dense": np.zeros(HIDDEN, np.float32),
    }
    o = kernel(**ins)
    print("kernel output:", o.shape, o.dtype, float(np.abs(o).max()))

